# revision 26
# baseline (speedup 1.0000x reference)
"""AAM-Softmax loss on 8 Trainium2 NeuronCores.

Tensor-parallel over classes (C=100000 -> 12500/core, zero-padded to 12544).
Per core:
  - weight shard loaded f32 in 1536-class chunks (one DMA per chunk),
  - row norms via bn_stats + rsqrt by Newton iteration on the vector engine
    (keeps the scalar engine's activation-table set fixed all kernel long),
  - normalize+bf16-cast (DVE), bounce normalized chunk to DRAM, one xbar
    DMA-transpose per (chunk, d-half) back to SBUF in [d, c] layout,
  - bf16 matmuls vs emb^T -> cos(theta) in PSUM [128b x 1536c],
  - one fused ScalarE pass per PSUM tile: sigmoid(30*cos - 30) with
    accum_out row-sum.  e^30*sigmoid(30(x-1)) == min(e^(30x), e^30) up to a
    smooth kink at x=1 (|rel err on the final loss| ~1e-3, tolerance 2e-2);
    this implements the reference's clip(cos, -1, 1) upper clip; the lower
    clip contributes < 1e-8 relative and is dropped,
  - partial sums AllReduced across cores (8 x [512] f32),
  - target-class correction from host-gathered weight[labels] (replicated):
    t = clip(<emb_b, wlab_b>/||wlab_b||), cos(th+m) = t cos m - sqrt(1-t^2) sin m,
    sqrt via Newton-rsqrt; label/margin terms use the same sigmoid form,
  - loss = mean(ln(e^30*(S - sig_t + sig_m) - n_pad) - 30*marg) on device.
"""

import sys

if "/opt/trn_rl_repo" not in sys.path:
    sys.path.insert(0, "/opt/trn_rl_repo")

import math

import numpy as np

B, D, C = 512, 256, 100000
N_CORES = 8
C_PER = C // N_CORES            # 12500
C_PAD = 12544                   # 98 tiles of 128
N_PAD_TOTAL = float((C_PAD - C_PER) * N_CORES)   # 352 zero rows -> ~1.0 each
MARGIN = 0.2
SCALE = 30.0
E30 = float(np.exp(30.0))
COS_M = float(math.cos(MARGIN))
SIN_M = float(math.sin(MARGIN))
W_CHUNK = 3072
W_PSUM = 1536
CHUNKS = [256, 512, 1024, 3072, 3072, 3072, 1280, 256]  # sums to 12544
N_BBLK = 4
MAGIC = 0x5F3759DF

_PROGRAM = None


def _newton_rsqrt(nc, sb, mybir, x_ap, out_ap, ncols, tag):
    """out = x^-0.5 elementwise on [128, ncols] via bit-trick + 3 Newton iters.

    Runs entirely on the vector engine.  x = 0 yields a large finite value
    (so 0-padded weight rows normalize to 0 without NaN).
    """
    f32 = mybir.dt.float32
    i32 = mybir.dt.int32
    u32 = mybir.dt.uint32
    AL = mybir.AluOpType
    sh = [128, 24]
    u = sb.tile(sh, u32, tag=f"nw_u{tag}", name=f"nw_u{tag}")
    t2 = sb.tile(sh, i32, tag=f"nw_t2{tag}", name=f"nw_t2{tag}")
    t3 = sb.tile(sh, i32, tag=f"nw_t3{tag}", name=f"nw_t3{tag}")
    y = sb.tile(sh, f32, tag=f"nw_y{tag}", name=f"nw_y{tag}")
    yy = sb.tile(sh, f32, tag=f"nw_yy{tag}", name=f"nw_yy{tag}")
    c = slice(0, ncols)
    nc.vector.tensor_scalar(out=u[:, c], in0=x_ap.bitcast(u32), scalar1=1,
                            scalar2=None, op0=AL.logical_shift_right)
    nc.vector.tensor_scalar(out=t2[:, c], in0=u[:, c].bitcast(i32),
                            scalar1=MAGIC, scalar2=None, op0=AL.subtract)
    nc.vector.tensor_scalar(out=t3[:, c], in0=t2[:, c], scalar1=-1,
                            scalar2=None, op0=AL.mult)
    cur = t3[:, c].bitcast(f32)
    for _ in range(2):
        nc.vector.tensor_tensor(out=yy[:, c], in0=cur, in1=cur, op=AL.mult)
        nc.vector.tensor_tensor(out=yy[:, c], in0=yy[:, c], in1=x_ap, op=AL.mult)
        nc.vector.tensor_scalar(out=yy[:, c], in0=yy[:, c], scalar1=-0.5,
                                scalar2=1.5, op0=AL.mult, op1=AL.add)
        nc.vector.tensor_tensor(out=y[:, c], in0=cur, in1=yy[:, c], op=AL.mult)
        cur = y[:, c]
    nc.vector.tensor_copy(out_ap, y[:, c])


def _build_program(chunks=None, do_collective=True):
    from concourse import bacc, mybir, tile

    f32 = mybir.dt.float32
    bf16 = mybir.dt.bfloat16
    AL = mybir.AluOpType
    ACT = mybir.ActivationFunctionType

    if chunks is None:
        chunks = CHUNKS
    nc = bacc.Bacc(num_devices=N_CORES)

    w_ext = nc.dram_tensor("w", [128, C_PAD // 128, D], f32, kind="ExternalInput")
    embT_ext = nc.dram_tensor("embT", [D, B], f32, kind="ExternalInput")
    emb_ext = nc.dram_tensor("emb", [B, D], f32, kind="ExternalInput")
    wlab_ext = nc.dram_tensor("wlab", [B, D], f32, kind="ExternalInput")
    out_ext = nc.dram_tensor("out", [1, 1], f32, kind="ExternalOutput")

    with tile.TileContext(nc) as tc:
        with (
            tc.tile_pool(name="const", bufs=1) as cpool,
            tc.tile_pool(name="wpool", bufs=3) as wpool,
            tc.tile_pool(name="wnpool", bufs=3) as wnpool,
            tc.tile_pool(name="wntp", bufs=3) as wntp,
            tc.tile_pool(name="epool", bufs=3) as epool,
            tc.tile_pool(name="stat", bufs=3) as stat,
            tc.tile_pool(name="psum", bufs=2, space="PSUM") as psum,
            tc.tile_pool(name="psfin", bufs=1, space="PSUM") as psfin,
            tc.tile_pool(name="dram", bufs=1, space="DRAM") as dram,
            tc.tile_pool(name="dramw", bufs=3, space="DRAM") as dramw,
        ):
            # ---- constants / replicated small inputs ----
            embT_f = [cpool.tile([128, B], f32, tag=f"embTf{h}", name=f"embTf{h}") for h in range(2)]
            embT_b = [cpool.tile([128, B], bf16, tag=f"embTb{h}", name=f"embTb{h}") for h in range(2)]
            for h in range(2):
                nc.scalar.dma_start(out=embT_f[h][:], in_=embT_ext[h * 128 : (h + 1) * 128, :])
                nc.vector.tensor_copy(embT_b[h][:], embT_f[h][:])

            emb_t = [cpool.tile([128, D], f32, tag=f"emb{b}", name=f"emb{b}") for b in range(N_BBLK)]
            wlab_t = [cpool.tile([128, D], f32, tag=f"wlab{b}", name=f"wlab{b}") for b in range(N_BBLK)]
            for b in range(N_BBLK):
                nc.scalar.dma_start(out=emb_t[b][:], in_=emb_ext[b * 128 : (b + 1) * 128, :])
                nc.scalar.dma_start(out=wlab_t[b][:], in_=wlab_ext[b * 128 : (b + 1) * 128, :])

            ones = cpool.tile([128, 1], f32, tag="ones")
            nc.vector.memset(ones[:], 1.0)
            bias_sig = cpool.tile([128, 1], f32, tag="bias_sig")
            nc.vector.memset(bias_sig[:], -SCALE)

            pcol = [cpool.tile([128, 16], f32, tag=f"pcol{b}", name=f"pcol{b}") for b in range(N_BBLK)]

            # ---- target-class values t = clip(cos(emb, w_lab), -1, 1) ----
            dotL = cpool.tile([128, N_BBLK], f32, tag="dotL")
            ssqL = cpool.tile([128, N_BBLK], f32, tag="ssqL")
            sqs = cpool.tile([128, D], f32, tag="sqs")
            for b in range(N_BBLK):
                nc.vector.tensor_tensor(out=sqs[:], in0=emb_t[b][:], in1=wlab_t[b][:], op=AL.mult)
                nc.vector.reduce_sum(dotL[:, b : b + 1], sqs[:], axis=mybir.AxisListType.X)
                nc.vector.tensor_tensor(out=sqs[:], in0=wlab_t[b][:], in1=wlab_t[b][:], op=AL.mult)
                nc.vector.reduce_sum(ssqL[:, b : b + 1], sqs[:], axis=mybir.AxisListType.X)
            rinvL = cpool.tile([128, N_BBLK], f32, tag="rinvL")
            _newton_rsqrt(nc, cpool, mybir, ssqL[:], rinvL[:], N_BBLK, "L")
            tq = cpool.tile([128, N_BBLK], f32, tag="tq")
            nc.vector.tensor_tensor(out=tq[:], in0=dotL[:], in1=rinvL[:], op=AL.mult)
            tcl = cpool.tile([128, N_BBLK], f32, tag="tcl")
            nc.vector.tensor_scalar(
                out=tcl[:], in0=tq[:], scalar1=1.0, scalar2=-1.0, op0=AL.min, op1=AL.max,
            )

            ccA_in = dram.tile([128, N_BBLK], f32, name="ccA_in")
            ccA_out = dram.tile([N_CORES, 128, N_BBLK], f32, addr_space="Shared", name="ccA_out")
            SgA = cpool.tile([128, N_BBLK, N_CORES], f32, tag="SgA")

            # ---- main loop over class chunks ----
            ct0 = 0
            pci = 0
            for ci, W in enumerate(chunks):
                ntile = W // 128
                w_big = wpool.tile([128, 24, D], f32, tag="w", name=f"w_{ci}")
                nc.scalar.dma_start(
                    out=w_big[:, :ntile, :], in_=w_ext[:, ct0 : ct0 + ntile, :]
                )
                bn_c = stat.tile([128, 24, 6], f32, tag="bnc")
                ssq_c = stat.tile([128, 24], f32, tag="ssqc")
                for k in range(ntile):
                    nc.vector.bn_stats(bn_c[:, k, :], w_big[:, k, :])
                # ssq = cv_e + cv_o + (D/2) * (m_e^2 + m_o^2)
                ta = stat.tile([128, 24], f32, tag="ta")
                tb = stat.tile([128, 24], f32, tag="tb")
                nt = ntile
                nc.vector.tensor_tensor(out=ta[:, :nt], in0=bn_c[:, :nt, 1], in1=bn_c[:, :nt, 1], op=AL.mult)
                nc.vector.tensor_tensor(out=tb[:, :nt], in0=bn_c[:, :nt, 4], in1=bn_c[:, :nt, 4], op=AL.mult)
                nc.vector.tensor_tensor(out=ta[:, :nt], in0=ta[:, :nt], in1=tb[:, :nt], op=AL.add)
                nc.vector.tensor_tensor(out=tb[:, :nt], in0=bn_c[:, :nt, 2], in1=bn_c[:, :nt, 5], op=AL.add)
                nc.vector.tensor_scalar(out=ta[:, :nt], in0=ta[:, :nt], scalar1=float(D // 2), scalar2=1e-6, op0=AL.mult, op1=AL.add)
                nc.vector.tensor_tensor(out=ssq_c[:, :nt], in0=ta[:, :nt], in1=tb[:, :nt], op=AL.add)
                rinv_c = stat.tile([128, 24], f32, tag="rinvc")
                _newton_rsqrt(nc, stat, mybir, ssq_c[:, :nt], rinv_c[:, :nt], nt, "C")

                wn_big = wnpool.tile([128, 24, D], bf16, tag="wn", name=f"wn_{ci}")
                for k in range(ntile):
                    nc.vector.tensor_scalar(
                        out=wn_big[:, k, :], in0=w_big[:, k, :],
                        scalar1=rinv_c[:, k : k + 1], scalar2=None, op0=AL.mult,
                    )
                wnd = dramw.tile([ntile * 128, D], bf16, tag="wnd", name=f"wnd_{ci}")
                nc.sync.dma_start(
                    out=wnd[:].rearrange("(p k) d -> p k d", p=128),
                    in_=wn_big[:, :ntile, :],
                )
                wnT = [wntp.tile([128, W_CHUNK], bf16, tag=f"wnT{h}", name=f"wnT{h}_{ci}") for h in range(2)]
                for h in range(2):
                    nc.sync.dma_start_transpose(
                        out=wnT[h][:, :W], in_=wnd[:, h * 128 : (h + 1) * 128]
                    )
                for s0 in range(0, W, W_PSUM):
                    sW = min(W_PSUM, W - s0)
                    for b in range(N_BBLK):
                        ps = psum.tile([128, W_PSUM], f32, tag="ps")
                        for h in range(2):
                            for n0 in range(0, sW, 512):
                                n1 = min(n0 + 512, sW)
                                nc.tensor.matmul(
                                    ps[:, n0:n1],
                                    embT_b[h][:, b * 128 : (b + 1) * 128],
                                    wnT[h][:, s0 + n0 : s0 + n1],
                                    start=(h == 0),
                                    stop=(h == 1),
                                )
                        ex = epool.tile([128, W_PSUM], bf16, tag="ex")
                        nc.scalar.activation(
                            out=ex[:, :sW], in_=ps[:, :sW], func=ACT.Sigmoid,
                            scale=SCALE, bias=bias_sig[:],
                            accum_out=pcol[b][:, pci : pci + 1],
                        )
                    pci += 1
                ct0 += ntile
                if ci == len(chunks) - 2:
                    # fire the collective for everything so far; it overlaps
                    # the last chunk's compute
                    pciA = pci
                    PA = cpool.tile([128, N_BBLK], f32, tag="PA")
                    for b in range(N_BBLK):
                        nc.vector.reduce_sum(
                            PA[:, b : b + 1], pcol[b][:, :pciA], axis=mybir.AxisListType.X
                        )
                    nc.sync.dma_start(out=ccA_in[:], in_=PA[:])
                    if do_collective:
                        nc.gpsimd.collective_compute(
                            "AllGather",
                            mybir.AluOpType.bypass,
                            replica_groups=[list(range(N_CORES))],
                            ins=[ccA_in.opt()],
                            outs=[ccA_out.opt()],
                        )
                        nc.sync.dma_start(
                            out=SgA[:], in_=ccA_out[:].rearrange("r p f -> p f r")
                        )
                    else:
                        for r in range(N_CORES):
                            nc.sync.dma_start(out=SgA[:, :, r], in_=ccA_in[:])

            # ---- reduce partials and AllReduce across cores ----
            P = cpool.tile([128, N_BBLK], f32, tag="P")
            for b in range(N_BBLK):
                nc.vector.reduce_sum(
                    P[:, b : b + 1], pcol[b][:, pciA:pci], axis=mybir.AxisListType.X
                )
            cc_in = dram.tile([128, N_BBLK], f32)
            cc_out = dram.tile([N_CORES, 128, N_BBLK], f32, addr_space="Shared")
            nc.sync.dma_start(out=cc_in[:], in_=P[:])
            if do_collective:
                nc.gpsimd.collective_compute(
                    "AllGather",
                    mybir.AluOpType.bypass,
                    replica_groups=[list(range(N_CORES))],
                    ins=[cc_in.opt()],
                    outs=[cc_out.opt()],
                )
            Sg = cpool.tile([128, N_BBLK, N_CORES], f32, tag="Sg")
            if do_collective:
                nc.sync.dma_start(
                    out=Sg[:], in_=cc_out[:].rearrange("r p f -> p f r")
                )
            else:
                for r in range(N_CORES):
                    nc.sync.dma_start(out=Sg[:, :, r], in_=cc_in[:])
            SB = cpool.tile([128, N_BBLK], f32, tag="SB")
            nc.vector.reduce_sum(SB[:], Sg[:], axis=mybir.AxisListType.X)
            SA = cpool.tile([128, N_BBLK], f32, tag="SA")
            nc.vector.reduce_sum(SA[:], SgA[:], axis=mybir.AxisListType.X)
            S = cpool.tile([128, N_BBLK], f32, tag="S")
            nc.vector.tensor_tensor(out=S[:], in0=SA[:], in1=SB[:], op=AL.add)

            # ---- final: loss = mean(ln(E30*(S - sig_t + sig_m) - npad) - 30*marg) ----
            sig_t = cpool.tile([128, N_BBLK], f32, tag="sig_t")
            nc.scalar.activation(out=sig_t[:], in_=tcl[:], func=ACT.Sigmoid,
                                 scale=SCALE, bias=bias_sig[:])
            tsq = cpool.tile([128, N_BBLK], f32, tag="tsq")
            nc.vector.tensor_tensor(out=tsq[:], in0=tcl[:], in1=tcl[:], op=AL.mult)
            q = cpool.tile([128, N_BBLK], f32, tag="q")
            nc.vector.tensor_scalar(
                out=q[:], in0=tsq[:], scalar1=-1.0, scalar2=1.0, op0=AL.mult, op1=AL.add
            )
            qb = cpool.tile([128, N_BBLK], f32, tag="qb")
            nc.vector.tensor_scalar(out=qb[:], in0=q[:], scalar1=1e-20, scalar2=None, op0=AL.add)
            rq = cpool.tile([128, N_BBLK], f32, tag="rq")
            _newton_rsqrt(nc, cpool, mybir, qb[:], rq[:], N_BBLK, "Q")
            sroot = cpool.tile([128, N_BBLK], f32, tag="sroot")
            nc.vector.tensor_tensor(out=sroot[:], in0=q[:], in1=rq[:], op=AL.mult)
            m1 = cpool.tile([128, N_BBLK], f32, tag="m1")
            nc.vector.tensor_scalar(out=m1[:], in0=tcl[:], scalar1=COS_M, scalar2=None, op0=AL.mult)
            m2 = cpool.tile([128, N_BBLK], f32, tag="m2")
            nc.vector.tensor_scalar(out=m2[:], in0=sroot[:], scalar1=SIN_M, scalar2=None, op0=AL.mult)
            marg = cpool.tile([128, N_BBLK], f32, tag="marg")
            nc.vector.tensor_tensor(out=marg[:], in0=m1[:], in1=m2[:], op=AL.subtract)
            sig_m = cpool.tile([128, N_BBLK], f32, tag="sig_m")
            nc.scalar.activation(out=sig_m[:], in_=marg[:], func=ACT.Sigmoid,
                                 scale=SCALE, bias=bias_sig[:])
            m30 = cpool.tile([128, N_BBLK], f32, tag="m30")
            nc.vector.tensor_scalar(out=m30[:], in0=marg[:], scalar1=SCALE, scalar2=None, op0=AL.mult)
            d0 = cpool.tile([128, N_BBLK], f32, tag="d0")
            nc.vector.tensor_tensor(out=d0[:], in0=S[:], in1=sig_t[:], op=AL.subtract)
            nc.vector.tensor_tensor(out=d0[:], in0=d0[:], in1=sig_m[:], op=AL.add)
            d1 = cpool.tile([128, N_BBLK], f32, tag="d1")
            nc.vector.tensor_scalar(
                out=d1[:], in0=d0[:], scalar1=E30, scalar2=-N_PAD_TOTAL, op0=AL.mult, op1=AL.add
            )
            lse = cpool.tile([128, N_BBLK], f32, tag="lse")
            nc.scalar.activation(out=lse[:], in_=d1[:], func=ACT.Ln)
            Lb = cpool.tile([128, N_BBLK], f32, tag="Lb")
            nc.vector.tensor_tensor(out=Lb[:], in0=lse[:], in1=m30[:], op=AL.subtract)
            red = cpool.tile([128, 1], f32, tag="red")
            nc.vector.reduce_sum(red[:], Lb[:], axis=mybir.AxisListType.X)
            fin = psfin.tile([1, 1], f32, tag="fin")
            nc.tensor.matmul(fin[:], ones[:], red[:], start=True, stop=True)
            loss_sb = cpool.tile([1, 1], f32, tag="loss")
            nc.scalar.activation(out=loss_sb[:], in_=fin[:], func=ACT.Copy, scale=1.0 / B)
            nc.sync.dma_start(out=out_ext[:], in_=loss_sb[:])

    nc.finalize()
    return nc


def _get_program():
    global _PROGRAM
    if _PROGRAM is None:
        _PROGRAM = _build_program()
    return _PROGRAM


def kernel(embeddings, weight, labels):
    from concourse.bass_utils import run_bass_kernel_spmd

    embeddings = np.asarray(embeddings, dtype=np.float32)
    weight = np.asarray(weight, dtype=np.float32)
    labels = np.asarray(labels)

    embT = np.ascontiguousarray(embeddings.T)
    wlab = np.ascontiguousarray(weight[labels])
    w_pad = np.zeros((N_CORES, C_PAD, D), dtype=np.float32)
    w_pad[:, :C_PER] = weight.reshape(N_CORES, C_PER, D)
    # partition-major: w_pm[i][p, k, :] = shard_i[k*128 + p]
    w_pad = np.ascontiguousarray(
        w_pad.reshape(N_CORES, C_PAD // 128, 128, D).transpose(0, 2, 1, 3)
    )

    in_maps = [
        {"w": w_pad[i], "embT": embT, "emb": embeddings, "wlab": wlab}
        for i in range(N_CORES)
    ]
    nc = _get_program()
    res = run_bass_kernel_spmd(nc, in_maps, core_ids=list(range(N_CORES)))
    return np.asarray(res.results[0]["out"][0, 0], dtype=np.float32)


# revision 27
# speedup vs baseline: 1.1599x; 1.1599x over previous
"""AAM-Softmax loss on 8 Trainium2 NeuronCores.

Tensor-parallel over classes (C=100000 -> 12500/core, zero-padded to 12544).
Per core:
  - weight shard loaded f32 in 1536-class chunks (one DMA per chunk),
  - row norms via bn_stats + rsqrt by Newton iteration on the vector engine
    (keeps the scalar engine's activation-table set fixed all kernel long),
  - normalize+bf16-cast (DVE), bounce normalized chunk to DRAM, one xbar
    DMA-transpose per (chunk, d-half) back to SBUF in [d, c] layout,
  - bf16 matmuls vs emb^T -> cos(theta) in PSUM [128b x 1536c],
  - one fused ScalarE pass per PSUM tile: sigmoid(30*cos - 30) with
    accum_out row-sum.  e^30*sigmoid(30(x-1)) == min(e^(30x), e^30) up to a
    smooth kink at x=1 (|rel err on the final loss| ~1e-3, tolerance 2e-2);
    this implements the reference's clip(cos, -1, 1) upper clip; the lower
    clip contributes < 1e-8 relative and is dropped,
  - partial sums AllReduced across cores (8 x [512] f32),
  - target-class correction from host-gathered weight[labels] (replicated):
    t = clip(<emb_b, wlab_b>/||wlab_b||), cos(th+m) = t cos m - sqrt(1-t^2) sin m,
    sqrt via Newton-rsqrt; label/margin terms use the same sigmoid form,
  - loss = mean(ln(e^30*(S - sig_t + sig_m) - n_pad) - 30*marg) on device.
"""

import sys

if "/opt/trn_rl_repo" not in sys.path:
    sys.path.insert(0, "/opt/trn_rl_repo")

import math

import numpy as np

B, D, C = 512, 256, 100000
N_CORES = 8
C_PER = C // N_CORES            # 12500
C_PAD = 12544                   # 98 tiles of 128
N_PAD_TOTAL = float((C_PAD - C_PER) * N_CORES)   # 352 zero rows -> ~1.0 each
MARGIN = 0.2
SCALE = 30.0
E30 = float(np.exp(30.0))
COS_M = float(math.cos(MARGIN))
SIN_M = float(math.sin(MARGIN))
W_CHUNK = 3072
W_PSUM = 1536
CHUNKS = [256, 512, 1024, 3072, 3072, 3072, 1280, 256]  # sums to 12544
N_BBLK = 4
MAGIC = 0x5F3759DF

_PROGRAM = None


def _newton_rsqrt(nc, sb, mybir, x_ap, out_ap, ncols, tag):
    """out = x^-0.5 elementwise on [128, ncols] via bit-trick + 3 Newton iters.

    Runs entirely on the vector engine.  x = 0 yields a large finite value
    (so 0-padded weight rows normalize to 0 without NaN).
    """
    f32 = mybir.dt.float32
    i32 = mybir.dt.int32
    u32 = mybir.dt.uint32
    AL = mybir.AluOpType
    sh = [128, 24]
    u = sb.tile(sh, u32, tag=f"nw_u{tag}", name=f"nw_u{tag}")
    t2 = sb.tile(sh, i32, tag=f"nw_t2{tag}", name=f"nw_t2{tag}")
    t3 = sb.tile(sh, i32, tag=f"nw_t3{tag}", name=f"nw_t3{tag}")
    y = sb.tile(sh, f32, tag=f"nw_y{tag}", name=f"nw_y{tag}")
    yy = sb.tile(sh, f32, tag=f"nw_yy{tag}", name=f"nw_yy{tag}")
    c = slice(0, ncols)
    nc.vector.tensor_scalar(out=u[:, c], in0=x_ap.bitcast(u32), scalar1=1,
                            scalar2=None, op0=AL.logical_shift_right)
    nc.vector.tensor_scalar(out=t2[:, c], in0=u[:, c].bitcast(i32),
                            scalar1=MAGIC, scalar2=None, op0=AL.subtract)
    nc.vector.tensor_scalar(out=t3[:, c], in0=t2[:, c], scalar1=-1,
                            scalar2=None, op0=AL.mult)
    cur = t3[:, c].bitcast(f32)
    for _ in range(2):
        nc.vector.tensor_tensor(out=yy[:, c], in0=cur, in1=cur, op=AL.mult)
        nc.vector.tensor_tensor(out=yy[:, c], in0=yy[:, c], in1=x_ap, op=AL.mult)
        nc.vector.tensor_scalar(out=yy[:, c], in0=yy[:, c], scalar1=-0.5,
                                scalar2=1.5, op0=AL.mult, op1=AL.add)
        nc.vector.tensor_tensor(out=y[:, c], in0=cur, in1=yy[:, c], op=AL.mult)
        cur = y[:, c]
    nc.vector.tensor_copy(out_ap, y[:, c])


def _build_program(chunks=None, do_collective=True):
    from concourse import bacc, mybir, tile

    f32 = mybir.dt.float32
    bf16 = mybir.dt.bfloat16
    AL = mybir.AluOpType
    ACT = mybir.ActivationFunctionType

    if chunks is None:
        chunks = CHUNKS
    nc = bacc.Bacc(num_devices=N_CORES)

    w_ext = nc.dram_tensor("w", [128, C_PAD // 128, D], f32, kind="ExternalInput")
    embT_ext = nc.dram_tensor("embT", [D, B], f32, kind="ExternalInput")
    emb_ext = nc.dram_tensor("emb", [B, D], f32, kind="ExternalInput")
    wlab_ext = nc.dram_tensor("wlab", [B, D], f32, kind="ExternalInput")
    out_ext = nc.dram_tensor("out", [1, 1], f32, kind="ExternalOutput")

    with tile.TileContext(nc) as tc:
        with (
            tc.tile_pool(name="const", bufs=1) as cpool,
            tc.tile_pool(name="wpool", bufs=3) as wpool,
            tc.tile_pool(name="wnpool", bufs=3) as wnpool,
            tc.tile_pool(name="wntp", bufs=3) as wntp,
            tc.tile_pool(name="epool", bufs=3) as epool,
            tc.tile_pool(name="stat", bufs=3) as stat,
            tc.tile_pool(name="psum", bufs=2, space="PSUM") as psum,
            tc.tile_pool(name="psfin", bufs=1, space="PSUM") as psfin,
            tc.tile_pool(name="dram", bufs=1, space="DRAM") as dram,
            tc.tile_pool(name="dramw", bufs=3, space="DRAM") as dramw,
        ):
            # ---- constants / replicated small inputs ----
            embT_f = [cpool.tile([128, B], f32, tag=f"embTf{h}", name=f"embTf{h}") for h in range(2)]
            embT_b = [cpool.tile([128, B], bf16, tag=f"embTb{h}", name=f"embTb{h}") for h in range(2)]
            for h in range(2):
                nc.scalar.dma_start(out=embT_f[h][:], in_=embT_ext[h * 128 : (h + 1) * 128, :])
                nc.vector.tensor_copy(embT_b[h][:], embT_f[h][:])

            emb_t = [cpool.tile([128, D], f32, tag=f"emb{b}", name=f"emb{b}") for b in range(N_BBLK)]
            wlab_t = [cpool.tile([128, D], f32, tag=f"wlab{b}", name=f"wlab{b}") for b in range(N_BBLK)]
            for b in range(N_BBLK):
                nc.scalar.dma_start(out=emb_t[b][:], in_=emb_ext[b * 128 : (b + 1) * 128, :])
                nc.scalar.dma_start(out=wlab_t[b][:], in_=wlab_ext[b * 128 : (b + 1) * 128, :])

            ones = cpool.tile([128, 1], f32, tag="ones")
            nc.vector.memset(ones[:], 1.0)
            bias_sig = cpool.tile([128, 1], f32, tag="bias_sig")
            nc.vector.memset(bias_sig[:], -SCALE)

            pcol = [cpool.tile([128, 16], f32, tag=f"pcol{b}", name=f"pcol{b}") for b in range(N_BBLK)]

            # ---- target-class values t = clip(cos(emb, w_lab), -1, 1) ----
            dotL = cpool.tile([128, N_BBLK], f32, tag="dotL")
            ssqL = cpool.tile([128, N_BBLK], f32, tag="ssqL")
            sqs = cpool.tile([128, D], f32, tag="sqs")
            for b in range(N_BBLK):
                nc.vector.tensor_tensor(out=sqs[:], in0=emb_t[b][:], in1=wlab_t[b][:], op=AL.mult)
                nc.vector.reduce_sum(dotL[:, b : b + 1], sqs[:], axis=mybir.AxisListType.X)
                nc.vector.tensor_tensor(out=sqs[:], in0=wlab_t[b][:], in1=wlab_t[b][:], op=AL.mult)
                nc.vector.reduce_sum(ssqL[:, b : b + 1], sqs[:], axis=mybir.AxisListType.X)
            rinvL = cpool.tile([128, N_BBLK], f32, tag="rinvL")
            _newton_rsqrt(nc, cpool, mybir, ssqL[:], rinvL[:], N_BBLK, "L")
            tq = cpool.tile([128, N_BBLK], f32, tag="tq")
            nc.vector.tensor_tensor(out=tq[:], in0=dotL[:], in1=rinvL[:], op=AL.mult)
            tcl = cpool.tile([128, N_BBLK], f32, tag="tcl")
            nc.vector.tensor_scalar(
                out=tcl[:], in0=tq[:], scalar1=1.0, scalar2=-1.0, op0=AL.min, op1=AL.max,
            )

            # ---- main loop over class chunks ----
            ct0 = 0
            pci = 0
            for ci, W in enumerate(chunks):
                ntile = W // 128
                w_big = wpool.tile([128, 24, D], f32, tag="w", name=f"w_{ci}")
                nc.scalar.dma_start(
                    out=w_big[:, :ntile, :], in_=w_ext[:, ct0 : ct0 + ntile, :]
                )
                bn_c = stat.tile([128, 24, 6], f32, tag="bnc")
                ssq_c = stat.tile([128, 24], f32, tag="ssqc")
                for k in range(ntile):
                    nc.vector.bn_stats(bn_c[:, k, :], w_big[:, k, :])
                # ssq = cv_e + cv_o + (D/2) * (m_e^2 + m_o^2)
                ta = stat.tile([128, 24], f32, tag="ta")
                tb = stat.tile([128, 24], f32, tag="tb")
                nt = ntile
                nc.vector.tensor_tensor(out=ta[:, :nt], in0=bn_c[:, :nt, 1], in1=bn_c[:, :nt, 1], op=AL.mult)
                nc.vector.tensor_tensor(out=tb[:, :nt], in0=bn_c[:, :nt, 4], in1=bn_c[:, :nt, 4], op=AL.mult)
                nc.vector.tensor_tensor(out=ta[:, :nt], in0=ta[:, :nt], in1=tb[:, :nt], op=AL.add)
                nc.vector.tensor_tensor(out=tb[:, :nt], in0=bn_c[:, :nt, 2], in1=bn_c[:, :nt, 5], op=AL.add)
                nc.vector.tensor_scalar(out=ta[:, :nt], in0=ta[:, :nt], scalar1=float(D // 2), scalar2=1e-6, op0=AL.mult, op1=AL.add)
                nc.vector.tensor_tensor(out=ssq_c[:, :nt], in0=ta[:, :nt], in1=tb[:, :nt], op=AL.add)
                rinv_c = stat.tile([128, 24], f32, tag="rinvc")
                _newton_rsqrt(nc, stat, mybir, ssq_c[:, :nt], rinv_c[:, :nt], nt, "C")

                wn_big = wnpool.tile([128, 24, D], bf16, tag="wn", name=f"wn_{ci}")
                for k in range(ntile):
                    nc.vector.tensor_scalar(
                        out=wn_big[:, k, :], in0=w_big[:, k, :],
                        scalar1=rinv_c[:, k : k + 1], scalar2=None, op0=AL.mult,
                    )
                wnd = dramw.tile([ntile * 128, D], bf16, tag="wnd", name=f"wnd_{ci}")
                nc.sync.dma_start(
                    out=wnd[:].rearrange("(p k) d -> p k d", p=128),
                    in_=wn_big[:, :ntile, :],
                )
                wnT = [wntp.tile([128, W_CHUNK], bf16, tag=f"wnT{h}", name=f"wnT{h}_{ci}") for h in range(2)]
                for h in range(2):
                    nc.sync.dma_start_transpose(
                        out=wnT[h][:, :W], in_=wnd[:, h * 128 : (h + 1) * 128]
                    )
                for s0 in range(0, W, W_PSUM):
                    sW = min(W_PSUM, W - s0)
                    for b in range(N_BBLK):
                        ps = psum.tile([128, W_PSUM], f32, tag="ps")
                        for h in range(2):
                            for n0 in range(0, sW, 512):
                                n1 = min(n0 + 512, sW)
                                nc.tensor.matmul(
                                    ps[:, n0:n1],
                                    embT_b[h][:, b * 128 : (b + 1) * 128],
                                    wnT[h][:, s0 + n0 : s0 + n1],
                                    start=(h == 0),
                                    stop=(h == 1),
                                )
                        ex = epool.tile([128, W_PSUM], bf16, tag="ex")
                        nc.scalar.activation(
                            out=ex[:, :sW], in_=ps[:, :sW], func=ACT.Sigmoid,
                            scale=SCALE, bias=bias_sig[:],
                            accum_out=pcol[b][:, pci : pci + 1],
                        )
                    pci += 1
                ct0 += ntile

            # ---- reduce partials and AllReduce across cores ----
            P = cpool.tile([128, N_BBLK], f32, tag="P")
            for b in range(N_BBLK):
                nc.vector.reduce_sum(
                    P[:, b : b + 1], pcol[b][:, :pci], axis=mybir.AxisListType.X
                )
            cc_in = dram.tile([128, N_BBLK], f32)
            cc_out = dram.tile([N_CORES, 128, N_BBLK], f32, addr_space="Shared")
            nc.sync.dma_start(out=cc_in[:], in_=P[:])
            if do_collective:
                nc.gpsimd.collective_compute(
                    "AllGather",
                    mybir.AluOpType.bypass,
                    replica_groups=[list(range(N_CORES))],
                    ins=[cc_in.opt()],
                    outs=[cc_out.opt()],
                )
            Sg = cpool.tile([128, N_BBLK, N_CORES], f32, tag="Sg")
            if do_collective:
                nc.sync.dma_start(
                    out=Sg[:], in_=cc_out[:].rearrange("r p f -> p f r")
                )
            else:
                for r in range(N_CORES):
                    nc.sync.dma_start(out=Sg[:, :, r], in_=cc_in[:])
            S = cpool.tile([128, N_BBLK], f32, tag="S")
            nc.vector.reduce_sum(S[:], Sg[:], axis=mybir.AxisListType.X)

            # ---- final: loss = mean(ln(E30*(S - sig_t + sig_m) - npad) - 30*marg) ----
            sig_t = cpool.tile([128, N_BBLK], f32, tag="sig_t")
            nc.scalar.activation(out=sig_t[:], in_=tcl[:], func=ACT.Sigmoid,
                                 scale=SCALE, bias=bias_sig[:])
            tsq = cpool.tile([128, N_BBLK], f32, tag="tsq")
            nc.vector.tensor_tensor(out=tsq[:], in0=tcl[:], in1=tcl[:], op=AL.mult)
            q = cpool.tile([128, N_BBLK], f32, tag="q")
            nc.vector.tensor_scalar(
                out=q[:], in0=tsq[:], scalar1=-1.0, scalar2=1.0, op0=AL.mult, op1=AL.add
            )
            qb = cpool.tile([128, N_BBLK], f32, tag="qb")
            nc.vector.tensor_scalar(out=qb[:], in0=q[:], scalar1=1e-20, scalar2=None, op0=AL.add)
            rq = cpool.tile([128, N_BBLK], f32, tag="rq")
            _newton_rsqrt(nc, cpool, mybir, qb[:], rq[:], N_BBLK, "Q")
            sroot = cpool.tile([128, N_BBLK], f32, tag="sroot")
            nc.vector.tensor_tensor(out=sroot[:], in0=q[:], in1=rq[:], op=AL.mult)
            m1 = cpool.tile([128, N_BBLK], f32, tag="m1")
            nc.vector.tensor_scalar(out=m1[:], in0=tcl[:], scalar1=COS_M, scalar2=None, op0=AL.mult)
            m2 = cpool.tile([128, N_BBLK], f32, tag="m2")
            nc.vector.tensor_scalar(out=m2[:], in0=sroot[:], scalar1=SIN_M, scalar2=None, op0=AL.mult)
            marg = cpool.tile([128, N_BBLK], f32, tag="marg")
            nc.vector.tensor_tensor(out=marg[:], in0=m1[:], in1=m2[:], op=AL.subtract)
            sig_m = cpool.tile([128, N_BBLK], f32, tag="sig_m")
            nc.scalar.activation(out=sig_m[:], in_=marg[:], func=ACT.Sigmoid,
                                 scale=SCALE, bias=bias_sig[:])
            m30 = cpool.tile([128, N_BBLK], f32, tag="m30")
            nc.vector.tensor_scalar(out=m30[:], in0=marg[:], scalar1=SCALE, scalar2=None, op0=AL.mult)
            d0 = cpool.tile([128, N_BBLK], f32, tag="d0")
            nc.vector.tensor_tensor(out=d0[:], in0=S[:], in1=sig_t[:], op=AL.subtract)
            nc.vector.tensor_tensor(out=d0[:], in0=d0[:], in1=sig_m[:], op=AL.add)
            d1 = cpool.tile([128, N_BBLK], f32, tag="d1")
            nc.vector.tensor_scalar(
                out=d1[:], in0=d0[:], scalar1=E30, scalar2=-N_PAD_TOTAL, op0=AL.mult, op1=AL.add
            )
            lse = cpool.tile([128, N_BBLK], f32, tag="lse")
            nc.scalar.activation(out=lse[:], in_=d1[:], func=ACT.Ln)
            Lb = cpool.tile([128, N_BBLK], f32, tag="Lb")
            nc.vector.tensor_tensor(out=Lb[:], in0=lse[:], in1=m30[:], op=AL.subtract)
            red = cpool.tile([128, 1], f32, tag="red")
            nc.vector.reduce_sum(red[:], Lb[:], axis=mybir.AxisListType.X)
            fin = psfin.tile([1, 1], f32, tag="fin")
            nc.tensor.matmul(fin[:], ones[:], red[:], start=True, stop=True)
            loss_sb = cpool.tile([1, 1], f32, tag="loss")
            nc.scalar.activation(out=loss_sb[:], in_=fin[:], func=ACT.Copy, scale=1.0 / B)
            nc.sync.dma_start(out=out_ext[:], in_=loss_sb[:])

    nc.finalize()
    return nc


def _get_program():
    global _PROGRAM
    if _PROGRAM is None:
        _PROGRAM = _build_program()
    return _PROGRAM


def kernel(embeddings, weight, labels):
    from concourse.bass_utils import run_bass_kernel_spmd

    embeddings = np.asarray(embeddings, dtype=np.float32)
    weight = np.asarray(weight, dtype=np.float32)
    labels = np.asarray(labels)

    embT = np.ascontiguousarray(embeddings.T)
    wlab = np.ascontiguousarray(weight[labels])
    w_pad = np.zeros((N_CORES, C_PAD, D), dtype=np.float32)
    w_pad[:, :C_PER] = weight.reshape(N_CORES, C_PER, D)
    # partition-major: w_pm[i][p, k, :] = shard_i[k*128 + p]
    w_pad = np.ascontiguousarray(
        w_pad.reshape(N_CORES, C_PAD // 128, 128, D).transpose(0, 2, 1, 3)
    )

    in_maps = [
        {"w": w_pad[i], "embT": embT, "emb": embeddings, "wlab": wlab}
        for i in range(N_CORES)
    ]
    nc = _get_program()
    res = run_bass_kernel_spmd(nc, in_maps, core_ids=list(range(N_CORES)))
    return np.asarray(res.results[0]["out"][0, 0], dtype=np.float32)
